# revision 9
# baseline (speedup 1.0000x reference)
"""BatchedLightSimulation Trainium2 kernel.

Math: the two causal convolutions (scintillation 990 taps, SiPM impulse 990
taps) compose into one 1979-tap causal filter c.  Folding the sum-by-16
downsample in gives

    out[row, s] = gain[row] * sum_delta c16[delta] * u[row, 16*s + delta]

with c16[delta] = sum_{k=max(0,delta)}^{15} c[k - delta].  c decays like
exp(-l/15.3) so c16 truncated to delta >= -240 is exact at fp32 precision
(validated 4e-7 of absmax vs the jax reference).

Device mapping (per core, 4 ninputs = 192 (n,d) rows):
  polyphase m = 16q + r.  SBUF tile X[q, st, row, r] holds the 64B chunks
  u[row, 16q:16q+16].  For each output tile of 100 s-values and each phase
  r, matmuls accumulate W_r[q_rel, s_rel].T @ X[:, st, :, r] into
  psum[100, rows].  The q-window per s-tile is [s0-15, s0+112] (128
  partitions, rows >= 115 have zero weights); the time axis is padded by
  240 zeros so the window never underflows.  Epilogue: gain multiply
  (DVE), PE transpose to [row, s], DMA out.

Precision/perf choices, measured on TRN2:
  - fp32 matmuls self-load weights in 2 half-passes that cannot use the
    background weight buffer -> ~327 ns per matmul, LDW-serialized.
    Instead u and W are split hi/lo into bf16 (u = uhi + ulo exactly to
    ~2^-17) and each (st, r) does 3 bf16 matmuls (hi*hi, hi*lo, lo*hi;
    products are exact in fp32 PSUM).  End-to-end error vs the reference
    is 2.9e-6 of absmax (pure-fp32 path: 5e-7, fp32 noise floor).
  - The host ships each core's shard already split and in the
    [q, s-tile, row, r] polyphase layout (a pure permutation + dtype
    split done during the shard-and-copy step) so the input DMA is fully
    contiguous.  A 64B-chunk gather DMA measures ~38 GB/s vs ~300 GB/s
    contiguous, and SBUF tiles with 115 partitions DMA 3.3x slower than
    128-partition tiles, so q is padded to 128.
  - A dozen dummy matmuls on the identity run while x streams in so the
    PE HAM clock gate opens (1.2 -> 2.4 GHz) before the real matmuls.
"""

import numpy as np
import ml_dtypes

import concourse.bacc as bacc
import concourse.mybir as mybir
import concourse.tile as tile
from concourse.bass_utils import run_bass_kernel_spmd

# ---- problem constants (hardcoded per contract) ----
NINPUT, NDET, NTICK = 32, 48, 6400
NS = 16                    # downsample factor
S = NTICK // NS            # 400 output ticks
LIGHT_TICK = 0.1
CONV_TICKS = 990
NCORES = 8
N_PER_CORE = NINPUT // NCORES      # 4
ROWS = N_PER_CORE * NDET           # 192 rows per core
J = 15                             # q-steps of history (taps delta >= -16*J)
HALO = J
PAD = NS * HALO                    # 240 zero ticks prepended
TPAD = NTICK + PAD                 # 6640
STILE = 100                        # s-values per output tile
NST = S // STILE                   # 4
QW = STILE + HALO                  # 115 live q rows per tile
QP = 128                           # padded q partitions (DMA speed)
DMAX = NS * J                      # 240
N_WARM = 12                        # dummy matmuls to lift the HAM clock gate
XFREE = NST * ROWS * NS            # 12288
TALLOC = NS * STILE * (NST - 1) + NS * QP + NS   # 6848: strided-view extent

BF16 = ml_dtypes.bfloat16


def _build_taps(singlet_fraction_logit, log_tau_s, log_tau_t,
                light_oscillation_period, light_response_time):
    """c16[delta] for delta in [-DMAX, 15], float64."""
    dt = float(LIGHT_TICK)
    tt = np.arange(CONV_TICKS, dtype=np.float64)
    sf = 1.0 / (1.0 + np.exp(-float(singlet_fraction_logit)))
    tau_s = 10.0 ** float(log_tau_s)
    tau_t = 10.0 ** float(log_tau_t)
    per = float(light_oscillation_period)
    rt = float(light_response_time)
    p1 = sf * np.exp(-tt * dt / tau_s) * (1.0 - np.exp(-dt / tau_s))
    p3 = (1.0 - sf) * np.exp(-tt * dt / tau_t) * (1.0 - np.exp(-dt / tau_t))
    scint = p1 + p3
    t = tt * dt
    imp = np.exp(-t / rt) * np.sin(t / per)
    imp = imp / (per * rt * rt) * (per * per + rt * rt) * dt
    c = np.convolve(scint, imp)          # length 2*990-1 = 1979
    deltas = np.arange(-DMAX, 16)
    c16 = np.zeros(len(deltas), dtype=np.float64)
    for i, d in enumerate(deltas):
        ks = np.arange(max(0, d), 16)
        c16[i] = c[ks - d].sum()
    return c16                            # index i -> delta = i - DMAX


def _build_weights(c16):
    """W[q_rel, r, s_rel] float32 (QP rows, >=QW zero), shared by s-tiles."""
    w = np.zeros((QP, NS, STILE), dtype=np.float64)
    q_rel = np.arange(QP)[:, None, None]
    r = np.arange(NS)[None, :, None]
    s_rel = np.arange(STILE)[None, None, :]
    delta = 16 * (q_rel - HALO - s_rel) + r
    mask = (delta >= -DMAX) & (delta <= 15) & (q_rel < QW)
    w[mask] = c16[(delta + DMAX)[mask]]
    return np.ascontiguousarray(w, dtype=np.float32)


def _split_bf16(a):
    hi = a.astype(BF16)
    lo = (a - hi.astype(np.float32)).astype(BF16)
    return hi, lo


_PROGRAM = None


def _build_program():
    global _PROGRAM
    if _PROGRAM is not None:
        return _PROGRAM
    nc = bacc.Bacc("TRN2", target_bir_lowering=False, debug=False,
                   num_devices=NCORES)
    f32 = mybir.dt.float32
    bf16 = mybir.dt.bfloat16
    xh_d = nc.dram_tensor("xhi", [QP, XFREE], bf16, kind="ExternalInput")
    xl_d = nc.dram_tensor("xlo", [QP, XFREE], bf16, kind="ExternalInput")
    wh_d = nc.dram_tensor("whi", [QP, NS * STILE], bf16, kind="ExternalInput")
    wl_d = nc.dram_tensor("wlo", [QP, NS * STILE], bf16, kind="ExternalInput")
    g_d = nc.dram_tensor("gain", [128, ROWS], f32, kind="ExternalInput")
    i_d = nc.dram_tensor("ident", [128, 128], f32, kind="ExternalInput")
    o_d = nc.dram_tensor("out", [ROWS, S], f32, kind="ExternalOutput")

    HK = XFREE // 2          # half of x free extent (= s-tiles 0,1)

    with tile.TileContext(nc) as tc:
        with (
            tc.tile_pool(name="const", bufs=1) as cpool,
            tc.tile_pool(name="x", bufs=1) as xpool,
            tc.tile_pool(name="ep", bufs=2) as epool,
            tc.tile_pool(name="fin", bufs=1) as fpool,
            tc.tile_pool(name="ps", bufs=2, space="PSUM") as pspool,
            tc.tile_pool(name="warm", bufs=1, space="PSUM") as wpool,
            tc.tile_pool(name="psT", bufs=2, space="PSUM") as ptpool,
        ):
            # constants first on the scalar (ACT) HWDGE ring; x halves are
            # interleaved across both rings so s-tile 0/1 data lands early
            id_sb = cpool.tile([128, 128], f32, tag="id")
            nc.scalar.dma_start(id_sb[:], i_d[:])
            wh_sb = cpool.tile([QP, NS * STILE], bf16, tag="wh")
            nc.scalar.dma_start(wh_sb[:], wh_d[:])
            wl_sb = cpool.tile([QP, NS * STILE], bf16, tag="wl")
            nc.scalar.dma_start(wl_sb[:], wl_d[:])
            g_sb = cpool.tile([128, ROWS], f32, tag="g")
            nc.scalar.dma_start(g_sb[:], g_d[:])

            xh_sb = xpool.tile([QP, NST, ROWS, NS], bf16, tag="xh")
            xh_flat = xh_sb[:].rearrange("q st row r -> q (st row r)")
            xl_sb = xpool.tile([QP, NST, ROWS, NS], bf16, tag="xl")
            xl_flat = xl_sb[:].rearrange("q st row r -> q (st row r)")
            nc.sync.dma_start(xh_flat[:, 0:HK], xh_d[:, 0:HK])
            nc.scalar.dma_start(xl_flat[:, 0:HK], xl_d[:, 0:HK])
            nc.sync.dma_start(xh_flat[:, HK:XFREE], xh_d[:, HK:XFREE])
            nc.scalar.dma_start(xl_flat[:, HK:XFREE], xl_d[:, HK:XFREE])

            # PE warm-up: dummy bf16 matmuls on the identity keep TensorE
            # busy while x streams in, so the HAM clock gate opens before
            # the real matmuls start (1.2 -> 2.4 GHz).
            id_bf = id_sb[:].bitcast(bf16)            # [128, 256]
            ps_warm = wpool.tile([128, 256], f32, tag="warm")
            for _ in range(N_WARM):
                nc.tensor.matmul(ps_warm[:], id_bf[:, 0:128], id_bf[:],
                                 start=True, stop=True)

            fin_a = fpool.tile([128, S], f32, tag="fa")
            fin_b = fpool.tile([64, S], f32, tag="fb")

            for st in range(NST):
                ps = pspool.tile([STILE, ROWS], f32, tag="ps")
                for r in range(NS):
                    wh = wh_sb[:, r * STILE:(r + 1) * STILE]
                    wl = wl_sb[:, r * STILE:(r + 1) * STILE]
                    xh = xh_sb[:, st, :, r]
                    xl = xl_sb[:, st, :, r]
                    for k, (lhsT, rhs) in enumerate(
                            ((wh, xh), (wh, xl), (wl, xh))):
                        nc.tensor.matmul(
                            ps[:], lhsT, rhs,
                            start=(r == 0 and k == 0),
                            stop=(r == NS - 1 and k == 2),
                        )
                gained = epool.tile([STILE, ROWS], f32, tag="gained")
                nc.vector.tensor_mul(gained[:], ps[:], g_sb[0:STILE, :])
                pT_a = ptpool.tile([128, STILE], f32, tag="pTa")
                nc.tensor.transpose(pT_a[:], gained[:, 0:128],
                                    id_sb[0:STILE, 0:STILE])
                pT_b = ptpool.tile([64, STILE], f32, tag="pTb")
                nc.tensor.transpose(pT_b[:], gained[:, 128:ROWS],
                                    id_sb[0:STILE, 0:STILE])
                nc.vector.tensor_copy(fin_a[:, st * STILE:(st + 1) * STILE],
                                      pT_a[:])
                nc.vector.tensor_copy(fin_b[:, st * STILE:(st + 1) * STILE],
                                      pT_b[:])

            nc.sync.dma_start(o_d[0:128, :], fin_a[:])
            nc.sync.dma_start(o_d[128:ROWS, :], fin_b[:])

    nc.compile()
    _PROGRAM = nc
    return nc


def _prepare_inputs(timing_dist, singlet_fraction_logit, log_tau_s, log_tau_t,
                    light_oscillation_period, light_response_time, light_gain):
    u = np.ascontiguousarray(np.asarray(timing_dist, dtype=np.float32))
    assert u.shape == (NINPUT, NDET, NTICK)
    gain = np.asarray(light_gain, dtype=np.float32).reshape(NDET)

    c16 = _build_taps(singlet_fraction_logit, log_tau_s, log_tau_t,
                      light_oscillation_period, light_response_time)
    w = _build_weights(c16).reshape(QP, NS * STILE)
    whi, wlo = _split_bf16(w)

    gain_row = np.tile(gain, N_PER_CORE)                     # [ROWS]
    gain_rep = np.ascontiguousarray(
        np.broadcast_to(gain_row[None, :], (128, ROWS)), dtype=np.float32)
    ident = np.eye(128, dtype=np.float32)

    in_maps = []
    for c in range(NCORES):
        shard = u[c * N_PER_CORE:(c + 1) * N_PER_CORE].reshape(ROWS, NTICK)
        up = np.zeros((ROWS, TALLOC), dtype=np.float32)
        up[:, PAD:TPAD] = shard
        # polyphase relayout: x[q, st, row, r] = up[row, 1600*st + 16*q + r]
        xv = np.lib.stride_tricks.as_strided(
            up,
            shape=(QP, NST, ROWS, NS),
            strides=(NS * 4, NS * STILE * 4, up.strides[0], 4),
        )
        x = np.ascontiguousarray(xv, dtype=np.float32).reshape(QP, XFREE)
        xhi, xlo = _split_bf16(x)
        in_maps.append({"xhi": xhi, "xlo": xlo, "whi": whi, "wlo": wlo,
                        "gain": gain_rep, "ident": ident})
    return in_maps


def _run(in_maps, trace=False):
    nc = _build_program()
    res = run_bass_kernel_spmd(nc, in_maps, core_ids=list(range(NCORES)),
                               trace=trace)
    outs = [res.results[c]["out"].reshape(N_PER_CORE, NDET, S)
            for c in range(NCORES)]
    full = np.concatenate(outs, axis=0).astype(np.float32, copy=False)
    return full, res


def kernel(timing_dist, singlet_fraction_logit, log_tau_s, log_tau_t,
           light_oscillation_period, light_response_time, light_gain):
    in_maps = _prepare_inputs(
        timing_dist, singlet_fraction_logit, log_tau_s, log_tau_t,
        light_oscillation_period, light_response_time, light_gain)
    full, _ = _run(in_maps, trace=False)
    return full


# revision 11
# speedup vs baseline: 2.1312x; 2.1312x over previous
"""BatchedLightSimulation Trainium2 kernel.

Math: the two causal convolutions (scintillation 990 taps, SiPM impulse 990
taps) compose into one 1979-tap causal filter c.  Folding the sum-by-16
downsample in gives

    out[row, s] = gain[row] * sum_delta c16[delta] * u[row, 16*s + delta]

with c16[delta] = sum_{k=max(0,delta)}^{15} c[k - delta].  c decays like
exp(-l/15.3) so c16 truncated to delta >= -240 is exact at fp32 precision
(validated 4e-7 of absmax vs the jax reference).

Device mapping (per core, 4 ninputs = 192 (n,d) rows):
  polyphase m = 16q + r.  SBUF tile X[q, st, row, r] holds the 64B chunks
  u[row, 16q:16q+16].  For each output tile of 100 s-values and each phase
  r, matmuls accumulate W_r[q_rel, s_rel].T @ X[:, st, :, r] into
  psum[100, rows].  The q-window per s-tile is [s0-15, s0+112] (128
  partitions, rows >= 115 have zero weights); the time axis is padded by
  240 zeros so the window never underflows.  Epilogue: gain multiply
  (DVE), PE transpose to [row, s], DMA out.

Precision/perf choices, measured on TRN2:
  - fp32 matmuls self-load weights in 2 half-passes that cannot use the
    background weight buffer -> ~327 ns per matmul, LDW-serialized.
    Instead u and W are split hi/lo into bf16 (u = uhi + ulo exactly to
    ~2^-17) and each (st, r) does 3 bf16 matmuls (hi*hi, hi*lo, lo*hi;
    products are exact in fp32 PSUM).  End-to-end error vs the reference
    is 2.9e-6 of absmax (pure-fp32 path: 5e-7, fp32 noise floor).
  - The host ships each core's shard already split and in the
    [q, s-tile, row, r] polyphase layout (a pure permutation + dtype
    split done during the shard-and-copy step) so the input DMA is fully
    contiguous.  A 64B-chunk gather DMA measures ~38 GB/s vs ~300 GB/s
    contiguous, and SBUF tiles with 115 partitions DMA 3.3x slower than
    128-partition tiles, so q is padded to 128.
  - A dozen dummy matmuls on the identity run while x streams in so the
    PE HAM clock gate opens (1.2 -> 2.4 GHz) before the real matmuls.
"""

import numpy as np
import ml_dtypes

import concourse.bacc as bacc
import concourse.mybir as mybir
import concourse.tile as tile
from concourse.bass_utils import run_bass_kernel_spmd

# ---- problem constants (hardcoded per contract) ----
NINPUT, NDET, NTICK = 32, 48, 6400
NS = 16                    # downsample factor
S = NTICK // NS            # 400 output ticks
LIGHT_TICK = 0.1
CONV_TICKS = 990
NCORES = 8
N_PER_CORE = NINPUT // NCORES      # 4
ROWS = N_PER_CORE * NDET           # 192 rows per core
J = 15                             # q-steps of history (taps delta >= -16*J)
HALO = J
PAD = NS * HALO                    # 240 zero ticks prepended
TPAD = NTICK + PAD                 # 6640
STILE = 100                        # s-values per output tile
NST = S // STILE                   # 4
QW = STILE + HALO                  # 115 live q rows per tile
QP = 128                           # padded q partitions (DMA speed)
DMAX = NS * J                      # 240
N_WARM = 12                        # dummy matmuls to lift the HAM clock gate
WCOL = 128                         # weight columns (128 enables FWL)
XFREE = NST * ROWS * NS            # 12288
TALLOC = NS * STILE * (NST - 1) + NS * QP + NS   # 6848: strided-view extent

BF16 = ml_dtypes.bfloat16


def _build_taps(singlet_fraction_logit, log_tau_s, log_tau_t,
                light_oscillation_period, light_response_time):
    """c16[delta] for delta in [-DMAX, 15], float64."""
    dt = float(LIGHT_TICK)
    tt = np.arange(CONV_TICKS, dtype=np.float64)
    sf = 1.0 / (1.0 + np.exp(-float(singlet_fraction_logit)))
    tau_s = 10.0 ** float(log_tau_s)
    tau_t = 10.0 ** float(log_tau_t)
    per = float(light_oscillation_period)
    rt = float(light_response_time)
    p1 = sf * np.exp(-tt * dt / tau_s) * (1.0 - np.exp(-dt / tau_s))
    p3 = (1.0 - sf) * np.exp(-tt * dt / tau_t) * (1.0 - np.exp(-dt / tau_t))
    scint = p1 + p3
    t = tt * dt
    imp = np.exp(-t / rt) * np.sin(t / per)
    imp = imp / (per * rt * rt) * (per * per + rt * rt) * dt
    c = np.convolve(scint, imp)          # length 2*990-1 = 1979
    deltas = np.arange(-DMAX, 16)
    c16 = np.zeros(len(deltas), dtype=np.float64)
    for i, d in enumerate(deltas):
        ks = np.arange(max(0, d), 16)
        c16[i] = c[ks - d].sum()
    return c16                            # index i -> delta = i - DMAX


def _build_weights(c16):
    """W[q_rel, r, s_rel] float32 (QP rows, WCOL cols, zero-padded)."""
    w = np.zeros((QP, NS, WCOL), dtype=np.float64)
    q_rel = np.arange(QP)[:, None, None]
    r = np.arange(NS)[None, :, None]
    s_rel = np.arange(WCOL)[None, None, :]
    delta = 16 * (q_rel - HALO - s_rel) + r
    mask = ((delta >= -DMAX) & (delta <= 15) & (q_rel < QW)
            & (s_rel < STILE))
    w[mask] = c16[(delta + DMAX)[mask]]
    return np.ascontiguousarray(w, dtype=np.float32)


def _split_bf16(a):
    hi = a.astype(BF16)
    lo = (a - hi.astype(np.float32)).astype(BF16)
    return hi, lo


_PROGRAM = None


def _build_program():
    global _PROGRAM
    if _PROGRAM is not None:
        return _PROGRAM
    nc = bacc.Bacc("TRN2", target_bir_lowering=False, debug=False,
                   num_devices=NCORES)
    f32 = mybir.dt.float32
    bf16 = mybir.dt.bfloat16
    x_d = nc.dram_tensor("x", [QP, 2 * XFREE], bf16, kind="ExternalInput")
    wh_d = nc.dram_tensor("whi", [QP, NS * WCOL], bf16, kind="ExternalInput")
    wl_d = nc.dram_tensor("wlo", [QP, NS * WCOL], bf16, kind="ExternalInput")
    g_d = nc.dram_tensor("gain", [128, ROWS], f32, kind="ExternalInput")
    i_d = nc.dram_tensor("ident", [128, 128], f32, kind="ExternalInput")
    o_d = nc.dram_tensor("out", [ROWS, S], f32, kind="ExternalOutput")

    XQ = 2 * XFREE // NST    # one s-tile's x extent (hi+lo)

    with tile.TileContext(nc) as tc:
        with (
            tc.tile_pool(name="const", bufs=1) as cpool,
            tc.tile_pool(name="x", bufs=1) as xpool,
            tc.tile_pool(name="ep", bufs=2) as epool,
            tc.tile_pool(name="fin", bufs=1) as fpool,
            tc.tile_pool(name="ps", bufs=2, space="PSUM") as pspool,
            tc.tile_pool(name="warm", bufs=1, space="PSUM") as wpool,
            tc.tile_pool(name="psT", bufs=2, space="PSUM") as ptpool,
        ):
            # constants first on the scalar (ACT) HWDGE ring; x halves are
            # interleaved across both rings so s-tile 0/1 data lands early
            id_sb = cpool.tile([128, 128], f32, tag="id")
            nc.scalar.dma_start(id_sb[:], i_d[:])
            wh_sb = cpool.tile([QP, NS * WCOL], bf16, tag="wh")
            nc.scalar.dma_start(wh_sb[:], wh_d[:])
            wl_sb = cpool.tile([QP, NS * WCOL], bf16, tag="wl")
            nc.scalar.dma_start(wl_sb[:], wl_d[:])
            g_sb = cpool.tile([128, ROWS], f32, tag="g")
            nc.scalar.dma_start(g_sb[:], g_d[:])

            # x[q, st, r, h, row]: h in {hi, lo}; row contiguous so the
            # matmul moving operand streams stride-1.  One s-tile per DMA,
            # even s-tiles on the sync ring, odd on scalar (after consts).
            x_sb = xpool.tile([QP, NST, NS, 2, ROWS], bf16, tag="x")
            x_flat = x_sb[:].rearrange("q st r h row -> q (st r h row)")
            for st in range(NST):
                eng = nc.sync if st % 2 == 0 else nc.scalar
                eng.dma_start(x_flat[:, st * XQ:(st + 1) * XQ],
                              x_d[:, st * XQ:(st + 1) * XQ])

            # PE warm-up: dummy bf16 matmuls on the identity keep TensorE
            # busy while x streams in, so the HAM clock gate opens before
            # the real matmuls start (1.2 -> 2.4 GHz).
            id_bf = id_sb[:].bitcast(bf16)            # [128, 256]
            ps_warm = wpool.tile([128, 256], f32, tag="warm")
            for _ in range(N_WARM):
                nc.tensor.matmul(ps_warm[:], id_bf[:, 0:128], id_bf[:],
                                 start=True, stop=True)

            fin_a = fpool.tile([128, S], f32, tag="fa")
            fin_b = fpool.tile([64, S], f32, tag="fb")

            for st in range(NST):
                ps = pspool.tile([WCOL, 2 * ROWS], f32, tag="ps")
                for r in range(NS):
                    wh = wh_sb[:, r * WCOL:(r + 1) * WCOL]
                    wl = wl_sb[:, r * WCOL:(r + 1) * WCOL]
                    # whi @ [xhi | xlo] -> cols [0:192]=hi*hi, [192:384]=hi*lo
                    nc.tensor.matmul(
                        ps[:], wh, x_sb[:, st, r, :, :],
                        start=(r == 0), stop=False,
                    )
                    # wlo @ xhi accumulates onto cols [0:192]
                    nc.tensor.matmul(
                        ps[:, 0:ROWS], wl, x_sb[:, st, r, 0, :],
                        start=False, stop=(r == NS - 1),
                    )
                t_lo = epool.tile([STILE, ROWS], f32, tag="tlo")
                nc.vector.tensor_copy(t_lo[:], ps[0:STILE, ROWS:2 * ROWS])
                summed = epool.tile([STILE, ROWS], f32, tag="summed")
                nc.vector.tensor_add(summed[:], ps[0:STILE, 0:ROWS], t_lo[:])
                gained = epool.tile([STILE, ROWS], f32, tag="gained")
                nc.vector.tensor_mul(gained[:], summed[:], g_sb[0:STILE, :])
                pT_a = ptpool.tile([128, STILE], f32, tag="pTa")
                nc.tensor.transpose(pT_a[:], gained[:, 0:128],
                                    id_sb[0:STILE, 0:STILE])
                pT_b = ptpool.tile([64, STILE], f32, tag="pTb")
                nc.tensor.transpose(pT_b[:], gained[:, 128:ROWS],
                                    id_sb[0:STILE, 0:STILE])
                nc.vector.tensor_copy(fin_a[:, st * STILE:(st + 1) * STILE],
                                      pT_a[:])
                nc.vector.tensor_copy(fin_b[:, st * STILE:(st + 1) * STILE],
                                      pT_b[:])

            nc.sync.dma_start(o_d[0:128, :], fin_a[:])
            nc.sync.dma_start(o_d[128:ROWS, :], fin_b[:])

    nc.compile()
    _PROGRAM = nc
    return nc


def _prepare_inputs(timing_dist, singlet_fraction_logit, log_tau_s, log_tau_t,
                    light_oscillation_period, light_response_time, light_gain):
    u = np.ascontiguousarray(np.asarray(timing_dist, dtype=np.float32))
    assert u.shape == (NINPUT, NDET, NTICK)
    gain = np.asarray(light_gain, dtype=np.float32).reshape(NDET)

    c16 = _build_taps(singlet_fraction_logit, log_tau_s, log_tau_t,
                      light_oscillation_period, light_response_time)
    w = _build_weights(c16).reshape(QP, NS * WCOL)
    whi, wlo = _split_bf16(w)

    gain_row = np.tile(gain, N_PER_CORE)                     # [ROWS]
    gain_rep = np.ascontiguousarray(
        np.broadcast_to(gain_row[None, :], (128, ROWS)), dtype=np.float32)
    ident = np.eye(128, dtype=np.float32)

    in_maps = []
    for c in range(NCORES):
        shard = u[c * N_PER_CORE:(c + 1) * N_PER_CORE].reshape(ROWS, NTICK)
        up = np.zeros((ROWS, TALLOC), dtype=np.float32)
        up[:, PAD:TPAD] = shard
        # polyphase relayout: x[q, st, r, h, row] = split_h(
        #     up[row, 1600*st + 16*q + r])
        uphi, uplo = _split_bf16(up)
        xs = []
        for a in (uphi, uplo):
            xs.append(np.lib.stride_tricks.as_strided(
                a,
                shape=(QP, NST, NS, ROWS),
                strides=(NS * 2, NS * STILE * 2, 2, a.strides[0]),
            ))
        x = np.ascontiguousarray(np.stack(xs, axis=3)).reshape(QP, 2 * XFREE)
        in_maps.append({"x": x, "whi": whi, "wlo": wlo,
                        "gain": gain_rep, "ident": ident})
    return in_maps


def _run(in_maps, trace=False):
    nc = _build_program()
    res = run_bass_kernel_spmd(nc, in_maps, core_ids=list(range(NCORES)),
                               trace=trace)
    outs = [res.results[c]["out"].reshape(N_PER_CORE, NDET, S)
            for c in range(NCORES)]
    full = np.concatenate(outs, axis=0).astype(np.float32, copy=False)
    return full, res


def kernel(timing_dist, singlet_fraction_logit, log_tau_s, log_tau_t,
           light_oscillation_period, light_response_time, light_gain):
    in_maps = _prepare_inputs(
        timing_dist, singlet_fraction_logit, log_tau_s, log_tau_t,
        light_oscillation_period, light_response_time, light_gain)
    full, _ = _run(in_maps, trace=False)
    return full


# revision 12
# speedup vs baseline: 2.2508x; 1.0561x over previous
"""BatchedLightSimulation Trainium2 kernel.

Math: the two causal convolutions (scintillation 990 taps, SiPM impulse 990
taps) compose into one 1979-tap causal filter c.  Folding the sum-by-16
downsample in gives

    out[row, s] = gain[row] * sum_delta c16[delta] * u[row, 16*s + delta]

with c16[delta] = sum_{k=max(0,delta)}^{15} c[k - delta].  c decays like
exp(-l/15.3) so c16 truncated to delta >= -240 is exact at fp32 precision
(validated 4e-7 of absmax vs the jax reference).

Device mapping (per core, 4 ninputs = 192 (n,d) rows):
  polyphase m = 16q + r.  SBUF tile X[q, st, row, r] holds the 64B chunks
  u[row, 16q:16q+16].  For each output tile of 100 s-values and each phase
  r, matmuls accumulate W_r[q_rel, s_rel].T @ X[:, st, :, r] into
  psum[100, rows].  The q-window per s-tile is [s0-15, s0+112] (128
  partitions, rows >= 115 have zero weights); the time axis is padded by
  240 zeros so the window never underflows.  Epilogue: gain multiply
  (DVE), PE transpose to [row, s], DMA out.

Precision/perf choices, measured on TRN2:
  - fp32 matmuls self-load weights in 2 half-passes that cannot use the
    background weight buffer -> ~327 ns per matmul, LDW-serialized.
    Instead u and W are split hi/lo into bf16 (u = uhi + ulo exactly to
    ~2^-17) and each (st, r) does 3 bf16 matmuls (hi*hi, hi*lo, lo*hi;
    products are exact in fp32 PSUM).  End-to-end error vs the reference
    is 2.9e-6 of absmax (pure-fp32 path: 5e-7, fp32 noise floor).
  - The host ships each core's shard already split and in the
    [q, s-tile, row, r] polyphase layout (a pure permutation + dtype
    split done during the shard-and-copy step) so the input DMA is fully
    contiguous.  A 64B-chunk gather DMA measures ~38 GB/s vs ~300 GB/s
    contiguous, and SBUF tiles with 115 partitions DMA 3.3x slower than
    128-partition tiles, so q is padded to 128.
  - A dozen dummy matmuls on the identity run while x streams in so the
    PE HAM clock gate opens (1.2 -> 2.4 GHz) before the real matmuls.
"""

import numpy as np
import ml_dtypes

import concourse.bacc as bacc
import concourse.mybir as mybir
import concourse.tile as tile
from concourse.bass_utils import run_bass_kernel_spmd

# ---- problem constants (hardcoded per contract) ----
NINPUT, NDET, NTICK = 32, 48, 6400
NS = 16                    # downsample factor
S = NTICK // NS            # 400 output ticks
LIGHT_TICK = 0.1
CONV_TICKS = 990
NCORES = 8
N_PER_CORE = NINPUT // NCORES      # 4
ROWS = N_PER_CORE * NDET           # 192 rows per core
J = 15                             # q-steps of history (taps delta >= -16*J)
HALO = J
PAD = NS * HALO                    # 240 zero ticks prepended
TPAD = NTICK + PAD                 # 6640
STILE = 100                        # s-values per output tile
NST = S // STILE                   # 4
QW = STILE + HALO                  # 115 live q rows per tile
QP = 128                           # padded q partitions (DMA speed)
DMAX = NS * J                      # 240
N_WARM = 12                        # dummy matmuls to lift the HAM clock gate
WCOL = 128                         # weight columns (128 enables FWL)
XFREE = NST * ROWS * NS            # 12288
TALLOC = NS * STILE * (NST - 1) + NS * QP + NS   # 6848: strided-view extent

BF16 = ml_dtypes.bfloat16


def _build_taps(singlet_fraction_logit, log_tau_s, log_tau_t,
                light_oscillation_period, light_response_time):
    """c16[delta] for delta in [-DMAX, 15], float64."""
    dt = float(LIGHT_TICK)
    tt = np.arange(CONV_TICKS, dtype=np.float64)
    sf = 1.0 / (1.0 + np.exp(-float(singlet_fraction_logit)))
    tau_s = 10.0 ** float(log_tau_s)
    tau_t = 10.0 ** float(log_tau_t)
    per = float(light_oscillation_period)
    rt = float(light_response_time)
    p1 = sf * np.exp(-tt * dt / tau_s) * (1.0 - np.exp(-dt / tau_s))
    p3 = (1.0 - sf) * np.exp(-tt * dt / tau_t) * (1.0 - np.exp(-dt / tau_t))
    scint = p1 + p3
    t = tt * dt
    imp = np.exp(-t / rt) * np.sin(t / per)
    imp = imp / (per * rt * rt) * (per * per + rt * rt) * dt
    c = np.convolve(scint, imp)          # length 2*990-1 = 1979
    deltas = np.arange(-DMAX, 16)
    c16 = np.zeros(len(deltas), dtype=np.float64)
    for i, d in enumerate(deltas):
        ks = np.arange(max(0, d), 16)
        c16[i] = c[ks - d].sum()
    return c16                            # index i -> delta = i - DMAX


def _build_weights(c16):
    """W[q_rel, r, s_rel] float32 (QP rows, WCOL cols, zero-padded)."""
    w = np.zeros((QP, NS, WCOL), dtype=np.float64)
    q_rel = np.arange(QP)[:, None, None]
    r = np.arange(NS)[None, :, None]
    s_rel = np.arange(WCOL)[None, None, :]
    delta = 16 * (q_rel - HALO - s_rel) + r
    mask = ((delta >= -DMAX) & (delta <= 15) & (q_rel < QW)
            & (s_rel < STILE))
    w[mask] = c16[(delta + DMAX)[mask]]
    return np.ascontiguousarray(w, dtype=np.float32)


def _split_bf16(a):
    hi = a.astype(BF16)
    lo = (a - hi.astype(np.float32)).astype(BF16)
    return hi, lo


_PROGRAM = None


def _build_program():
    global _PROGRAM
    if _PROGRAM is not None:
        return _PROGRAM
    nc = bacc.Bacc("TRN2", target_bir_lowering=False, debug=False,
                   num_devices=NCORES)
    f32 = mybir.dt.float32
    bf16 = mybir.dt.bfloat16
    x_d = nc.dram_tensor("x", [QP, 2 * XFREE], bf16, kind="ExternalInput")
    wh_d = nc.dram_tensor("whi", [QP, NS * WCOL], bf16, kind="ExternalInput")
    wl_d = nc.dram_tensor("wlo", [QP, NS * WCOL], bf16, kind="ExternalInput")
    g_d = nc.dram_tensor("gain", [128, ROWS], f32, kind="ExternalInput")
    i_d = nc.dram_tensor("ident", [128, 128], f32, kind="ExternalInput")
    o_d = nc.dram_tensor("out", [ROWS, S], f32, kind="ExternalOutput")

    XQ = 2 * XFREE // NST    # one s-tile's x extent (hi+lo)

    with tile.TileContext(nc) as tc:
        with (
            tc.tile_pool(name="const", bufs=1) as cpool,
            tc.tile_pool(name="x", bufs=1) as xpool,
            tc.tile_pool(name="ep", bufs=2) as epool,
            tc.tile_pool(name="fin", bufs=1) as fpool,
            tc.tile_pool(name="ps", bufs=2, space="PSUM") as pspool,
            tc.tile_pool(name="warm", bufs=1, space="PSUM") as wpool,
            tc.tile_pool(name="psT", bufs=2, space="PSUM") as ptpool,
        ):
            # PE warm-up: dummy bf16 matmuls on a memset tile (no DMA
            # dependency) keep TensorE busy from ~2us so the HAM clock
            # gate opens (1.2 -> 2.4 GHz) before the real matmuls start.
            warm_w = cpool.tile([128, 256], bf16, tag="warmw")
            nc.vector.memset(warm_w[:], 1.0)
            ps_warm = wpool.tile([128, 256], f32, tag="warm")
            for _ in range(N_WARM):
                nc.tensor.matmul(ps_warm[:], warm_w[:, 0:128], warm_w[:],
                                 start=True, stop=True)

            # weights first on the sync ring (it starts earliest), then
            # even s-tile x chunks; gain/ident + odd s-tiles on scalar.
            wh_sb = cpool.tile([QP, NS * WCOL], bf16, tag="wh")
            nc.sync.dma_start(wh_sb[:], wh_d[:])
            wl_sb = cpool.tile([QP, NS * WCOL], bf16, tag="wl")
            nc.sync.dma_start(wl_sb[:], wl_d[:])
            g_sb = cpool.tile([128, ROWS], f32, tag="g")
            nc.scalar.dma_start(g_sb[:], g_d[:])
            id_sb = cpool.tile([128, 128], f32, tag="id")
            nc.scalar.dma_start(id_sb[:], i_d[:])

            # x[q, st, r, h, row]: h in {hi, lo}; row contiguous so the
            # matmul moving operand streams stride-1.  Half s-tile per DMA
            # (8 r-phases), even s-tiles on sync, odd on scalar, so the
            # first matmuls are gated on 1/8th of x, not half of it.
            x_sb = xpool.tile([QP, NST, NS, 2, ROWS], bf16, tag="x")
            x_flat = x_sb[:].rearrange("q st r h row -> q (st r h row)")
            for st in (0, 2, 1, 3):
                eng = nc.sync if st % 2 == 0 else nc.scalar
                for hh in range(2):
                    lo = st * XQ + hh * XQ // 2
                    eng.dma_start(x_flat[:, lo:lo + XQ // 2],
                                  x_d[:, lo:lo + XQ // 2])

            fin_a = fpool.tile([128, S], f32, tag="fa")
            fin_b = fpool.tile([64, S], f32, tag="fb")

            for st in range(NST):
                ps = pspool.tile([WCOL, 2 * ROWS], f32, tag="ps")
                for r in range(NS):
                    wh = wh_sb[:, r * WCOL:(r + 1) * WCOL]
                    wl = wl_sb[:, r * WCOL:(r + 1) * WCOL]
                    # whi @ [xhi | xlo] -> cols [0:192]=hi*hi, [192:384]=hi*lo
                    nc.tensor.matmul(
                        ps[:], wh, x_sb[:, st, r, :, :],
                        start=(r == 0), stop=False,
                    )
                    # wlo @ xhi accumulates onto cols [0:192]
                    nc.tensor.matmul(
                        ps[:, 0:ROWS], wl, x_sb[:, st, r, 0, :],
                        start=False, stop=(r == NS - 1),
                    )
                t_lo = epool.tile([STILE, ROWS], f32, tag="tlo")
                nc.vector.tensor_copy(t_lo[:], ps[0:STILE, ROWS:2 * ROWS])
                summed = epool.tile([STILE, ROWS], f32, tag="summed")
                nc.vector.tensor_add(summed[:], ps[0:STILE, 0:ROWS], t_lo[:])
                gained = epool.tile([STILE, ROWS], f32, tag="gained")
                nc.vector.tensor_mul(gained[:], summed[:], g_sb[0:STILE, :])
                pT_a = ptpool.tile([128, STILE], f32, tag="pTa")
                nc.tensor.transpose(pT_a[:], gained[:, 0:128],
                                    id_sb[0:STILE, 0:STILE])
                pT_b = ptpool.tile([64, STILE], f32, tag="pTb")
                nc.tensor.transpose(pT_b[:], gained[:, 128:ROWS],
                                    id_sb[0:STILE, 0:STILE])
                nc.vector.tensor_copy(fin_a[:, st * STILE:(st + 1) * STILE],
                                      pT_a[:])
                nc.vector.tensor_copy(fin_b[:, st * STILE:(st + 1) * STILE],
                                      pT_b[:])

            nc.sync.dma_start(o_d[0:128, :], fin_a[:])
            nc.sync.dma_start(o_d[128:ROWS, :], fin_b[:])

    nc.compile()
    _PROGRAM = nc
    return nc


def _prepare_inputs(timing_dist, singlet_fraction_logit, log_tau_s, log_tau_t,
                    light_oscillation_period, light_response_time, light_gain):
    u = np.ascontiguousarray(np.asarray(timing_dist, dtype=np.float32))
    assert u.shape == (NINPUT, NDET, NTICK)
    gain = np.asarray(light_gain, dtype=np.float32).reshape(NDET)

    c16 = _build_taps(singlet_fraction_logit, log_tau_s, log_tau_t,
                      light_oscillation_period, light_response_time)
    w = _build_weights(c16).reshape(QP, NS * WCOL)
    whi, wlo = _split_bf16(w)

    gain_row = np.tile(gain, N_PER_CORE)                     # [ROWS]
    gain_rep = np.ascontiguousarray(
        np.broadcast_to(gain_row[None, :], (128, ROWS)), dtype=np.float32)
    ident = np.eye(128, dtype=np.float32)

    in_maps = []
    for c in range(NCORES):
        shard = u[c * N_PER_CORE:(c + 1) * N_PER_CORE].reshape(ROWS, NTICK)
        up = np.zeros((ROWS, TALLOC), dtype=np.float32)
        up[:, PAD:TPAD] = shard
        # polyphase relayout: x[q, st, r, h, row] = split_h(
        #     up[row, 1600*st + 16*q + r])
        uphi, uplo = _split_bf16(up)
        xs = []
        for a in (uphi, uplo):
            xs.append(np.lib.stride_tricks.as_strided(
                a,
                shape=(QP, NST, NS, ROWS),
                strides=(NS * 2, NS * STILE * 2, 2, a.strides[0]),
            ))
        x = np.ascontiguousarray(np.stack(xs, axis=3)).reshape(QP, 2 * XFREE)
        in_maps.append({"x": x, "whi": whi, "wlo": wlo,
                        "gain": gain_rep, "ident": ident})
    return in_maps


def _run(in_maps, trace=False):
    nc = _build_program()
    res = run_bass_kernel_spmd(nc, in_maps, core_ids=list(range(NCORES)),
                               trace=trace)
    outs = [res.results[c]["out"].reshape(N_PER_CORE, NDET, S)
            for c in range(NCORES)]
    full = np.concatenate(outs, axis=0).astype(np.float32, copy=False)
    return full, res


def kernel(timing_dist, singlet_fraction_logit, log_tau_s, log_tau_t,
           light_oscillation_period, light_response_time, light_gain):
    in_maps = _prepare_inputs(
        timing_dist, singlet_fraction_logit, log_tau_s, log_tau_t,
        light_oscillation_period, light_response_time, light_gain)
    full, _ = _run(in_maps, trace=False)
    return full


# revision 13
# speedup vs baseline: 2.2871x; 1.0161x over previous
"""BatchedLightSimulation Trainium2 kernel.

Math: the two causal convolutions (scintillation 990 taps, SiPM impulse 990
taps) compose into one 1979-tap causal filter c.  Folding the sum-by-16
downsample in gives

    out[row, s] = gain[row] * sum_delta c16[delta] * u[row, 16*s + delta]

with c16[delta] = sum_{k=max(0,delta)}^{15} c[k - delta].  c decays like
exp(-l/15.3) so c16 truncated to delta >= -240 is exact at fp32 precision
(validated 4e-7 of absmax vs the jax reference).

Device mapping (per core, 4 ninputs = 192 (n,d) rows):
  polyphase m = 16q + r.  SBUF tile X[q, st, row, r] holds the 64B chunks
  u[row, 16q:16q+16].  For each output tile of 100 s-values and each phase
  r, matmuls accumulate W_r[q_rel, s_rel].T @ X[:, st, :, r] into
  psum[100, rows].  The q-window per s-tile is [s0-15, s0+112] (128
  partitions, rows >= 115 have zero weights); the time axis is padded by
  240 zeros so the window never underflows.  Epilogue: gain multiply
  (DVE), PE transpose to [row, s], DMA out.

Precision/perf choices, measured on TRN2:
  - fp32 matmuls self-load weights in 2 half-passes that cannot use the
    background weight buffer -> ~327 ns per matmul, LDW-serialized.
    Instead u and W are split hi/lo into bf16 (u = uhi + ulo exactly to
    ~2^-17) and each (st, r) does 3 bf16 matmuls (hi*hi, hi*lo, lo*hi;
    products are exact in fp32 PSUM).  End-to-end error vs the reference
    is 2.9e-6 of absmax (pure-fp32 path: 5e-7, fp32 noise floor).
  - The host ships each core's shard already split and in the
    [q, s-tile, row, r] polyphase layout (a pure permutation + dtype
    split done during the shard-and-copy step) so the input DMA is fully
    contiguous.  A 64B-chunk gather DMA measures ~38 GB/s vs ~300 GB/s
    contiguous, and SBUF tiles with 115 partitions DMA 3.3x slower than
    128-partition tiles, so q is padded to 128.
  - A dozen dummy matmuls on the identity run while x streams in so the
    PE HAM clock gate opens (1.2 -> 2.4 GHz) before the real matmuls.
"""

import numpy as np
import ml_dtypes

import concourse.bacc as bacc
import concourse.mybir as mybir
import concourse.tile as tile
from concourse.bass_utils import run_bass_kernel_spmd

# ---- problem constants (hardcoded per contract) ----
NINPUT, NDET, NTICK = 32, 48, 6400
NS = 16                    # downsample factor
S = NTICK // NS            # 400 output ticks
LIGHT_TICK = 0.1
CONV_TICKS = 990
NCORES = 8
N_PER_CORE = NINPUT // NCORES      # 4
ROWS = N_PER_CORE * NDET           # 192 rows per core
J = 15                             # q-steps of history (taps delta >= -16*J)
HALO = J
PAD = NS * HALO                    # 240 zero ticks prepended
TPAD = NTICK + PAD                 # 6640
STILE = 100                        # s-values per output tile
NST = S // STILE                   # 4
QW = STILE + HALO                  # 115 live q rows per tile
QP = 128                           # padded q partitions (DMA speed)
DMAX = NS * J                      # 240
N_WARM = 34                        # dummy matmuls to lift the HAM clock gate
WCOL = 128                         # weight columns (128 enables FWL)
XFREE = NST * ROWS * NS            # 12288
TALLOC = NS * STILE * (NST - 1) + NS * QP + NS   # 6848: strided-view extent

BF16 = ml_dtypes.bfloat16


def _build_taps(singlet_fraction_logit, log_tau_s, log_tau_t,
                light_oscillation_period, light_response_time):
    """c16[delta] for delta in [-DMAX, 15], float64."""
    dt = float(LIGHT_TICK)
    tt = np.arange(CONV_TICKS, dtype=np.float64)
    sf = 1.0 / (1.0 + np.exp(-float(singlet_fraction_logit)))
    tau_s = 10.0 ** float(log_tau_s)
    tau_t = 10.0 ** float(log_tau_t)
    per = float(light_oscillation_period)
    rt = float(light_response_time)
    p1 = sf * np.exp(-tt * dt / tau_s) * (1.0 - np.exp(-dt / tau_s))
    p3 = (1.0 - sf) * np.exp(-tt * dt / tau_t) * (1.0 - np.exp(-dt / tau_t))
    scint = p1 + p3
    t = tt * dt
    imp = np.exp(-t / rt) * np.sin(t / per)
    imp = imp / (per * rt * rt) * (per * per + rt * rt) * dt
    c = np.convolve(scint, imp)          # length 2*990-1 = 1979
    deltas = np.arange(-DMAX, 16)
    c16 = np.zeros(len(deltas), dtype=np.float64)
    for i, d in enumerate(deltas):
        ks = np.arange(max(0, d), 16)
        c16[i] = c[ks - d].sum()
    return c16                            # index i -> delta = i - DMAX


def _build_weights(c16):
    """W[q_rel, r, s_rel] float32 (QP rows, WCOL cols, zero-padded)."""
    w = np.zeros((QP, NS, WCOL), dtype=np.float64)
    q_rel = np.arange(QP)[:, None, None]
    r = np.arange(NS)[None, :, None]
    s_rel = np.arange(WCOL)[None, None, :]
    delta = 16 * (q_rel - HALO - s_rel) + r
    mask = ((delta >= -DMAX) & (delta <= 15) & (q_rel < QW)
            & (s_rel < STILE))
    w[mask] = c16[(delta + DMAX)[mask]]
    return np.ascontiguousarray(w, dtype=np.float32)


def _split_bf16(a):
    hi = a.astype(BF16)
    lo = (a - hi.astype(np.float32)).astype(BF16)
    return hi, lo


_PROGRAM = None


def _build_program():
    global _PROGRAM
    if _PROGRAM is not None:
        return _PROGRAM
    nc = bacc.Bacc("TRN2", target_bir_lowering=False, debug=False,
                   num_devices=NCORES)
    f32 = mybir.dt.float32
    bf16 = mybir.dt.bfloat16
    x_d = nc.dram_tensor("x", [QP, 2 * XFREE], bf16, kind="ExternalInput")
    wh_d = nc.dram_tensor("whi", [QP, NS * WCOL], bf16, kind="ExternalInput")
    wl_d = nc.dram_tensor("wlo", [QP, NS * WCOL], bf16, kind="ExternalInput")
    g_d = nc.dram_tensor("gain", [128, ROWS], f32, kind="ExternalInput")
    i_d = nc.dram_tensor("ident", [128, 128], f32, kind="ExternalInput")
    o_d = nc.dram_tensor("out", [ROWS, S], f32, kind="ExternalOutput")

    XQ = 2 * XFREE // NST    # one s-tile's x extent (hi+lo)

    with tile.TileContext(nc) as tc:
        with (
            tc.tile_pool(name="const", bufs=1) as cpool,
            tc.tile_pool(name="x", bufs=1) as xpool,
            tc.tile_pool(name="ep", bufs=2) as epool,
            tc.tile_pool(name="fin", bufs=1) as fpool,
            tc.tile_pool(name="ps", bufs=3, space="PSUM") as pspool,
            tc.tile_pool(name="warm", bufs=1, space="PSUM") as wpool,
            tc.tile_pool(name="psT", bufs=2, space="PSUM") as ptpool,
        ):
            # PE warm-up: dummy bf16 matmuls on a memset tile (no DMA
            # dependency) keep TensorE busy from ~2us so the HAM clock
            # gate opens (1.2 -> 2.4 GHz) before the real matmuls start.
            warm_w = cpool.tile([128, 256], bf16, tag="warmw")
            nc.vector.memset(warm_w[:], 1.0)
            ps_warm = wpool.tile([128, 256], f32, tag="warm")
            for _ in range(N_WARM):
                nc.tensor.matmul(ps_warm[:], warm_w[:, 0:128], warm_w[:],
                                 start=True, stop=True)

            # one weight tensor first on each ring, then x chunks; the
            # first matmuls are gated on whi + the first x chunk only.
            wh_sb = cpool.tile([QP, NS * WCOL], bf16, tag="wh")
            nc.sync.dma_start(wh_sb[:], wh_d[:])
            wl_sb = cpool.tile([QP, NS * WCOL], bf16, tag="wl")
            nc.scalar.dma_start(wl_sb[:], wl_d[:])
            g_sb = cpool.tile([128, ROWS], f32, tag="g")
            nc.scalar.dma_start(g_sb[:], g_d[:])
            id_sb = cpool.tile([128, 128], f32, tag="id")
            nc.scalar.dma_start(id_sb[:], i_d[:])

            # x[q, st, r, h, row]: h in {hi, lo}; row contiguous so the
            # matmul moving operand streams stride-1.  Half s-tile per DMA
            # (8 r-phases), even s-tiles on sync, odd on scalar, so the
            # first matmuls are gated on 1/8th of x, not half of it.
            x_sb = xpool.tile([QP, NST, NS, 2, ROWS], bf16, tag="x")
            x_flat = x_sb[:].rearrange("q st r h row -> q (st r h row)")
            for st in (0, 2, 1, 3):
                eng = nc.sync if st % 2 == 0 else nc.scalar
                for hh in range(2):
                    lo = st * XQ + hh * XQ // 2
                    eng.dma_start(x_flat[:, lo:lo + XQ // 2],
                                  x_d[:, lo:lo + XQ // 2])

            fin_a = fpool.tile([128, S], f32, tag="fa")
            fin_b = fpool.tile([64, S], f32, tag="fb")

            for st in range(NST):
                ps = pspool.tile([WCOL, 2 * ROWS], f32, tag="ps")
                for r in range(NS):
                    wh = wh_sb[:, r * WCOL:(r + 1) * WCOL]
                    wl = wl_sb[:, r * WCOL:(r + 1) * WCOL]
                    # whi @ [xhi | xlo] -> cols [0:192]=hi*hi, [192:384]=hi*lo
                    nc.tensor.matmul(
                        ps[:], wh, x_sb[:, st, r, :, :],
                        start=(r == 0), stop=False,
                    )
                    # wlo @ xhi accumulates onto cols [0:192]
                    nc.tensor.matmul(
                        ps[:, 0:ROWS], wl, x_sb[:, st, r, 0, :],
                        start=False, stop=(r == NS - 1),
                    )
                t_lo = epool.tile([STILE, ROWS], f32, tag="tlo")
                nc.vector.tensor_copy(t_lo[:], ps[0:STILE, ROWS:2 * ROWS])
                summed = epool.tile([STILE, ROWS], f32, tag="summed")
                nc.vector.tensor_add(summed[:], ps[0:STILE, 0:ROWS], t_lo[:])
                gained = epool.tile([STILE, ROWS], f32, tag="gained")
                nc.vector.tensor_mul(gained[:], summed[:], g_sb[0:STILE, :])
                pT_a = ptpool.tile([128, STILE], f32, tag="pTa")
                nc.tensor.transpose(pT_a[:], gained[:, 0:128],
                                    id_sb[0:STILE, 0:STILE])
                pT_b = ptpool.tile([64, STILE], f32, tag="pTb")
                nc.tensor.transpose(pT_b[:], gained[:, 128:ROWS],
                                    id_sb[0:STILE, 0:STILE])
                nc.vector.tensor_copy(fin_a[:, st * STILE:(st + 1) * STILE],
                                      pT_a[:])
                nc.vector.tensor_copy(fin_b[:, st * STILE:(st + 1) * STILE],
                                      pT_b[:])

            nc.sync.dma_start(o_d[0:128, :], fin_a[:])
            nc.sync.dma_start(o_d[128:ROWS, :], fin_b[:])

    nc.compile()
    _PROGRAM = nc
    return nc


def _prepare_inputs(timing_dist, singlet_fraction_logit, log_tau_s, log_tau_t,
                    light_oscillation_period, light_response_time, light_gain):
    u = np.ascontiguousarray(np.asarray(timing_dist, dtype=np.float32))
    assert u.shape == (NINPUT, NDET, NTICK)
    gain = np.asarray(light_gain, dtype=np.float32).reshape(NDET)

    c16 = _build_taps(singlet_fraction_logit, log_tau_s, log_tau_t,
                      light_oscillation_period, light_response_time)
    w = _build_weights(c16).reshape(QP, NS * WCOL)
    whi, wlo = _split_bf16(w)

    gain_row = np.tile(gain, N_PER_CORE)                     # [ROWS]
    gain_rep = np.ascontiguousarray(
        np.broadcast_to(gain_row[None, :], (128, ROWS)), dtype=np.float32)
    ident = np.eye(128, dtype=np.float32)

    in_maps = []
    for c in range(NCORES):
        shard = u[c * N_PER_CORE:(c + 1) * N_PER_CORE].reshape(ROWS, NTICK)
        up = np.zeros((ROWS, TALLOC), dtype=np.float32)
        up[:, PAD:TPAD] = shard
        # polyphase relayout: x[q, st, r, h, row] = split_h(
        #     up[row, 1600*st + 16*q + r])
        uphi, uplo = _split_bf16(up)
        xs = []
        for a in (uphi, uplo):
            xs.append(np.lib.stride_tricks.as_strided(
                a,
                shape=(QP, NST, NS, ROWS),
                strides=(NS * 2, NS * STILE * 2, 2, a.strides[0]),
            ))
        x = np.ascontiguousarray(np.stack(xs, axis=3)).reshape(QP, 2 * XFREE)
        in_maps.append({"x": x, "whi": whi, "wlo": wlo,
                        "gain": gain_rep, "ident": ident})
    return in_maps


def _run(in_maps, trace=False):
    nc = _build_program()
    res = run_bass_kernel_spmd(nc, in_maps, core_ids=list(range(NCORES)),
                               trace=trace)
    outs = [res.results[c]["out"].reshape(N_PER_CORE, NDET, S)
            for c in range(NCORES)]
    full = np.concatenate(outs, axis=0).astype(np.float32, copy=False)
    return full, res


def kernel(timing_dist, singlet_fraction_logit, log_tau_s, log_tau_t,
           light_oscillation_period, light_response_time, light_gain):
    in_maps = _prepare_inputs(
        timing_dist, singlet_fraction_logit, log_tau_s, log_tau_t,
        light_oscillation_period, light_response_time, light_gain)
    full, _ = _run(in_maps, trace=False)
    return full


# revision 15
# speedup vs baseline: 2.3459x; 1.0257x over previous
"""BatchedLightSimulation Trainium2 kernel.

Math: the two causal convolutions (scintillation 990 taps, SiPM impulse 990
taps) compose into one 1979-tap causal filter c.  Folding the sum-by-16
downsample in gives

    out[row, s] = gain[row] * sum_delta c16[delta] * u[row, 16*s + delta]

with c16[delta] = sum_{k=max(0,delta)}^{15} c[k - delta].  c decays like
exp(-l/15.3) so c16 truncated to delta >= -240 is exact at fp32 precision
(validated 4e-7 of absmax vs the jax reference).

Device mapping (per core, 4 ninputs = 192 (n,d) rows):
  polyphase m = 16q + r.  SBUF tile X[q, st, row, r] holds the 64B chunks
  u[row, 16q:16q+16].  For each output tile of 100 s-values and each phase
  r, matmuls accumulate W_r[q_rel, s_rel].T @ X[:, st, :, r] into
  psum[100, rows].  The q-window per s-tile is [s0-15, s0+112] (128
  partitions, rows >= 115 have zero weights); the time axis is padded by
  240 zeros so the window never underflows.  Epilogue: gain multiply
  (DVE), PE transpose to [row, s], DMA out.

Precision/perf choices, measured on TRN2:
  - fp32 matmuls self-load weights in 2 half-passes that cannot use the
    background weight buffer -> ~327 ns per matmul, LDW-serialized.
    Instead u and W are split hi/lo into bf16 (u = uhi + ulo exactly to
    ~2^-17) and each (st, r) does 3 bf16 matmuls (hi*hi, hi*lo, lo*hi;
    products are exact in fp32 PSUM).  End-to-end error vs the reference
    is 2.9e-6 of absmax (pure-fp32 path: 5e-7, fp32 noise floor).
  - The host ships each core's shard already split and in the
    [q, s-tile, row, r] polyphase layout (a pure permutation + dtype
    split done during the shard-and-copy step) so the input DMA is fully
    contiguous.  A 64B-chunk gather DMA measures ~38 GB/s vs ~300 GB/s
    contiguous, and SBUF tiles with 115 partitions DMA 3.3x slower than
    128-partition tiles, so q is padded to 128.
  - A dozen dummy matmuls on the identity run while x streams in so the
    PE HAM clock gate opens (1.2 -> 2.4 GHz) before the real matmuls.
"""

import numpy as np
import ml_dtypes

import concourse.bacc as bacc
import concourse.mybir as mybir
import concourse.tile as tile
from concourse.bass_utils import run_bass_kernel_spmd

# ---- problem constants (hardcoded per contract) ----
NINPUT, NDET, NTICK = 32, 48, 6400
NS = 16                    # downsample factor
S = NTICK // NS            # 400 output ticks
LIGHT_TICK = 0.1
CONV_TICKS = 990
NCORES = 8
N_PER_CORE = NINPUT // NCORES      # 4
ROWS = N_PER_CORE * NDET           # 192 rows per core
J = 15                             # q-steps of history (taps delta >= -16*J)
HALO = J
PAD = NS * HALO                    # 240 zero ticks prepended
TPAD = NTICK + PAD                 # 6640
STILE = 100                        # s-values per output tile
NST = S // STILE                   # 4
QW = STILE + HALO                  # 115 live q rows per tile
QP = 128                           # padded q partitions (DMA speed)
DMAX = NS * J                      # 240
N_WARM = 44                        # dummy matmuls to lift the HAM clock gate
WCOL = 128                         # weight columns (128 enables FWL)
XFREE = NST * ROWS * NS            # 12288
TALLOC = NS * STILE * (NST - 1) + NS * QP + NS   # 6848: strided-view extent

BF16 = ml_dtypes.bfloat16


def _build_taps(singlet_fraction_logit, log_tau_s, log_tau_t,
                light_oscillation_period, light_response_time):
    """c16[delta] for delta in [-DMAX, 15], float64."""
    dt = float(LIGHT_TICK)
    tt = np.arange(CONV_TICKS, dtype=np.float64)
    sf = 1.0 / (1.0 + np.exp(-float(singlet_fraction_logit)))
    tau_s = 10.0 ** float(log_tau_s)
    tau_t = 10.0 ** float(log_tau_t)
    per = float(light_oscillation_period)
    rt = float(light_response_time)
    p1 = sf * np.exp(-tt * dt / tau_s) * (1.0 - np.exp(-dt / tau_s))
    p3 = (1.0 - sf) * np.exp(-tt * dt / tau_t) * (1.0 - np.exp(-dt / tau_t))
    scint = p1 + p3
    t = tt * dt
    imp = np.exp(-t / rt) * np.sin(t / per)
    imp = imp / (per * rt * rt) * (per * per + rt * rt) * dt
    c = np.convolve(scint, imp)          # length 2*990-1 = 1979
    deltas = np.arange(-DMAX, 16)
    c16 = np.zeros(len(deltas), dtype=np.float64)
    for i, d in enumerate(deltas):
        ks = np.arange(max(0, d), 16)
        c16[i] = c[ks - d].sum()
    return c16                            # index i -> delta = i - DMAX


def _build_weights(c16):
    """W[q_rel, r, s_rel] float32 (QP rows, WCOL cols, zero-padded)."""
    w = np.zeros((QP, NS, WCOL), dtype=np.float64)
    q_rel = np.arange(QP)[:, None, None]
    r = np.arange(NS)[None, :, None]
    s_rel = np.arange(WCOL)[None, None, :]
    delta = 16 * (q_rel - HALO - s_rel) + r
    mask = ((delta >= -DMAX) & (delta <= 15) & (q_rel < QW)
            & (s_rel < STILE))
    w[mask] = c16[(delta + DMAX)[mask]]
    return np.ascontiguousarray(w, dtype=np.float32)


def _split_bf16(a):
    hi = a.astype(BF16)
    lo = (a - hi.astype(np.float32)).astype(BF16)
    return hi, lo


_PROGRAM = None


def _build_program():
    global _PROGRAM
    if _PROGRAM is not None:
        return _PROGRAM
    nc = bacc.Bacc("TRN2", target_bir_lowering=False, debug=False,
                   num_devices=NCORES)
    f32 = mybir.dt.float32
    bf16 = mybir.dt.bfloat16
    x_d = nc.dram_tensor("x", [QP, 2 * XFREE], bf16, kind="ExternalInput")
    wh_d = nc.dram_tensor("whi", [QP, NS * WCOL], bf16, kind="ExternalInput")
    wl_d = nc.dram_tensor("wlo", [QP, NS * WCOL], bf16, kind="ExternalInput")
    g_d = nc.dram_tensor("gain", [128, ROWS], f32, kind="ExternalInput")
    i_d = nc.dram_tensor("ident", [128, 128], f32, kind="ExternalInput")
    o_d = nc.dram_tensor("out", [ROWS, S], f32, kind="ExternalOutput")

    XQ = 2 * XFREE // NST    # one s-tile's x extent (hi+lo)

    with tile.TileContext(nc) as tc:
        with (
            tc.tile_pool(name="const", bufs=1) as cpool,
            tc.tile_pool(name="x", bufs=1) as xpool,
            tc.tile_pool(name="ep", bufs=2) as epool,
            tc.tile_pool(name="fin", bufs=1) as fpool,
            tc.tile_pool(name="ps", bufs=1, space="PSUM") as pspool,
            tc.tile_pool(name="warm", bufs=1, space="PSUM") as wpool,
            tc.tile_pool(name="psT", bufs=1, space="PSUM") as ptpool,
        ):
            # PE warm-up: dummy bf16 matmuls on a memset tile (no DMA
            # dependency) keep TensorE busy from ~2us so the HAM clock
            # gate opens (1.2 -> 2.4 GHz) before the real matmuls start.
            warm_w = cpool.tile([128, 256], bf16, tag="warmw")
            nc.vector.memset(warm_w[:], 1.0)
            ps_warm = wpool.tile([128, 256], f32, tag="warm")
            for _ in range(N_WARM):
                nc.tensor.matmul(ps_warm[:], warm_w[:, 0:128], warm_w[:],
                                 start=True, stop=True)

            # one weight tensor first on each ring, then x chunks; the
            # first matmuls are gated on whi + the first x chunk only.
            wh_sb = cpool.tile([QP, NS * WCOL], bf16, tag="wh")
            nc.sync.dma_start(wh_sb[:], wh_d[:])
            wl_sb = cpool.tile([QP, NS * WCOL], bf16, tag="wl")
            nc.scalar.dma_start(wl_sb[:], wl_d[:])

            # x[q, st, r, h, row]: h in {hi, lo}; row contiguous so the
            # matmul moving operand streams stride-1.  Half s-tile per DMA
            # (8 r-phases); each s-tile's halves go to different rings and
            # chunks are issued in consumption order, so the first matmuls
            # are gated on 1/8th of x and the stream stays fed.
            x_sb = xpool.tile([QP, NST, NS, 2, ROWS], bf16, tag="x")
            x_flat = x_sb[:].rearrange("q st r h row -> q (st r h row)")
            gi_done = False
            for st in range(NST):
                for hh in range(2):
                    eng = nc.sync if hh == 0 else nc.scalar
                    lo = st * XQ + hh * XQ // 2
                    eng.dma_start(x_flat[:, lo:lo + XQ // 2],
                                  x_d[:, lo:lo + XQ // 2])
                if st == 1 and not gi_done:
                    gi_done = True
                    g_sb = cpool.tile([128, ROWS], f32, tag="g")
                    nc.scalar.dma_start(g_sb[:], g_d[:])
                    id_sb = cpool.tile([128, 128], f32, tag="id")
                    nc.scalar.dma_start(id_sb[:], i_d[:])

            fin_a = fpool.tile([128, S], f32, tag="fa")
            fin_b = fpool.tile([64, S], f32, tag="fb")

            # all matmuls first (the PE-critical path), epilogues after:
            # Tile's scheduler then slots the transposes into PE gaps
            # instead of stalling the matmul stream at s-tile boundaries.
            ps_tiles = []
            for st in range(NST):
                ps = pspool.tile([WCOL, 2 * ROWS], f32, tag=f"ps{st}")
                ps_tiles.append(ps)
                for r in range(NS):
                    wh = wh_sb[:, r * WCOL:(r + 1) * WCOL]
                    wl = wl_sb[:, r * WCOL:(r + 1) * WCOL]
                    # whi @ [xhi | xlo] -> cols [0:192]=hi*hi, [192:384]=hi*lo
                    nc.tensor.matmul(
                        ps[:], wh, x_sb[:, st, r, :, :],
                        start=(r == 0), stop=False,
                    )
                    # wlo @ xhi accumulates onto cols [0:192]
                    nc.tensor.matmul(
                        ps[:, 0:ROWS], wl, x_sb[:, st, r, 0, :],
                        start=False, stop=(r == NS - 1),
                    )
            for st in range(NST):
                ps = ps_tiles[st]
                t_lo = epool.tile([STILE, ROWS], f32, tag="tlo")
                nc.vector.tensor_copy(t_lo[:], ps[0:STILE, ROWS:2 * ROWS])
                summed = epool.tile([STILE, ROWS], f32, tag="summed")
                nc.vector.tensor_add(summed[:], ps[0:STILE, 0:ROWS], t_lo[:])
                gained = epool.tile([STILE, ROWS], f32, tag="gained")
                nc.vector.tensor_mul(gained[:], summed[:], g_sb[0:STILE, :])
                pT_a = ptpool.tile([128, STILE], f32, tag="pTa")
                nc.tensor.transpose(pT_a[:], gained[:, 0:128],
                                    id_sb[0:STILE, 0:STILE])
                pT_b = ptpool.tile([64, STILE], f32, tag="pTb")
                nc.tensor.transpose(pT_b[:], gained[:, 128:ROWS],
                                    id_sb[0:STILE, 0:STILE])
                nc.vector.tensor_copy(fin_a[:, st * STILE:(st + 1) * STILE],
                                      pT_a[:])
                nc.vector.tensor_copy(fin_b[:, st * STILE:(st + 1) * STILE],
                                      pT_b[:])

            nc.sync.dma_start(o_d[0:128, :], fin_a[:])
            nc.sync.dma_start(o_d[128:ROWS, :], fin_b[:])

    nc.compile()
    _PROGRAM = nc
    return nc


def _prepare_inputs(timing_dist, singlet_fraction_logit, log_tau_s, log_tau_t,
                    light_oscillation_period, light_response_time, light_gain):
    u = np.ascontiguousarray(np.asarray(timing_dist, dtype=np.float32))
    assert u.shape == (NINPUT, NDET, NTICK)
    gain = np.asarray(light_gain, dtype=np.float32).reshape(NDET)

    c16 = _build_taps(singlet_fraction_logit, log_tau_s, log_tau_t,
                      light_oscillation_period, light_response_time)
    w = _build_weights(c16).reshape(QP, NS * WCOL)
    whi, wlo = _split_bf16(w)

    gain_row = np.tile(gain, N_PER_CORE)                     # [ROWS]
    gain_rep = np.ascontiguousarray(
        np.broadcast_to(gain_row[None, :], (128, ROWS)), dtype=np.float32)
    ident = np.eye(128, dtype=np.float32)

    in_maps = []
    for c in range(NCORES):
        shard = u[c * N_PER_CORE:(c + 1) * N_PER_CORE].reshape(ROWS, NTICK)
        up = np.zeros((ROWS, TALLOC), dtype=np.float32)
        up[:, PAD:TPAD] = shard
        # polyphase relayout: x[q, st, r, h, row] = split_h(
        #     up[row, 1600*st + 16*q + r])
        uphi, uplo = _split_bf16(up)
        xs = []
        for a in (uphi, uplo):
            xs.append(np.lib.stride_tricks.as_strided(
                a,
                shape=(QP, NST, NS, ROWS),
                strides=(NS * 2, NS * STILE * 2, 2, a.strides[0]),
            ))
        x = np.ascontiguousarray(np.stack(xs, axis=3)).reshape(QP, 2 * XFREE)
        in_maps.append({"x": x, "whi": whi, "wlo": wlo,
                        "gain": gain_rep, "ident": ident})
    return in_maps


def _run(in_maps, trace=False):
    nc = _build_program()
    res = run_bass_kernel_spmd(nc, in_maps, core_ids=list(range(NCORES)),
                               trace=trace)
    outs = [res.results[c]["out"].reshape(N_PER_CORE, NDET, S)
            for c in range(NCORES)]
    full = np.concatenate(outs, axis=0).astype(np.float32, copy=False)
    return full, res


def kernel(timing_dist, singlet_fraction_logit, log_tau_s, log_tau_t,
           light_oscillation_period, light_response_time, light_gain):
    in_maps = _prepare_inputs(
        timing_dist, singlet_fraction_logit, log_tau_s, log_tau_t,
        light_oscillation_period, light_response_time, light_gain)
    full, _ = _run(in_maps, trace=False)
    return full
